# revision 35
# baseline (speedup 1.0000x reference)
"""3-layer GAT encoder on 8 trn2 NeuronCores (Bass/Tile).

Strategy: edge-parallel sharding by destination node block (core k owns dst
nodes [k*6250, (k+1)*6250)), so all segment ops are core-local. Per layer the
aggregation is factored as out[n,h] = (sum_e w_e * hfeat[src_e]) / (sum_e w_e)
with w_e = exp(leaky_relu(al_s[src] + al_d[dst])). Per-edge work is done in
128-edge blocks: src features come from a dma_gather of 512-byte fp16 node
records (two table halves for int16 indices), al_d[dst] is broadcast via a
onehot matmul, and the segment sum is an edge-orientation onehot matmul
accumulated in PSUM per 128-dst-node window.

Perf notes vs the first version:
 - one-hots are built in DVE 2x/4x perf modes: oh2 (dst-part orientation) via
   tensor_scalar with a per-partition f32 iota scalar (single-src 4x); oh1
   (edge-part orientation) in (j-outer, k-inner) column order against a
   materialized iota pattern so both tensor_tensor operands are unit-stride.
 - records store features interleaved (f, h4) with heads padded to 4 so the
   per-edge weight multiply has both operands unit-stride (2x mode). Layer 3
   (1 head) instead folds w into the one-hot and streams the raw record as
   the matmul rhs.
 - the node table is split into two half tensors; each half's AllGather fires
   as soon as its windows finalize, overlapping the collective with the edge
   phase tail and the next layer's start. Node->table-row order is remapped
   (half-major, then rank-major) so AllGather's rank-major concat lands rows
   exactly where the gather indices expect them.
"""
import os
import numpy as np
from contextlib import ExitStack

import concourse.bass as bass
import concourse.bacc as bacc
import concourse.tile as tile
from concourse import mybir
from concourse.bass_utils import run_bass_kernel_spmd

F16 = mybir.dt.float16
F32 = mybir.dt.float32
I16 = mybir.dt.int16

N = 50000
NCORE = 8
NLOC = N // NCORE            # 6250
NWIN = (NLOC + 127) // 128   # 49
LASTW = NLOC - 128 * (NWIN - 1)  # 106
WINA = 25                    # windows in half A
ROWA = WINA * 128            # 3200 local rows in half A
ROWB = NLOC - ROWA           # 3050 local rows in half B
HALFA = NCORE * ROWA         # 25600 table rows in half A
HALFB = NCORE * ROWB         # 24400
H, F = 3, 43
NEG = 0.2
RECE1 = 128                  # f16 record L1: [(x6+1,h4)=28, pad, al_s f32@14:17]
RECE = 256                   # f16 record L2/L3 (512B)
K = 48                       # blocks per tile
JK = K * 128                 # one-hot cols per tile


def ap_of(t, offset_elems, dims):
    base = t if isinstance(t, bass.AP) else t[:]
    return bass.AP(tensor=base.tensor, offset=base.offset + offset_elems,
                   ap=[list(base.ap[0])] + [list(d) for d in dims])


def _remap_rows(src):
    """Global node id -> table row (half-major, rank-major, local)."""
    c = src // NLOC
    r = src - c * NLOC
    return np.where(r < ROWA, c * ROWA + r, HALFA + c * ROWB + (r - ROWA))


def _build_structure(src, dst):
    """Host: shard edges by dst core / 128-window / src half, uniform block
    structure across cores. Returns per-core upload arrays + schedule."""
    core = dst // NLOC
    dst_loc = dst - core * NLOC
    win = dst_loc // 128
    de = dst_loc % 128
    row = _remap_rows(src)
    half = (row >= HALFA).astype(np.int64)

    # bucket edges per (core, win, half)
    order = np.lexsort((half, win, core))
    rc, wc, hc, dec = row[order], win[order], half[order], de[order]
    key = ((core[order] * NWIN + wc) * 2 + hc)
    uniq, starts = np.unique(key, return_index=True)
    starts = list(starts) + [len(key)]
    counts = np.zeros((NCORE, NWIN, 2), np.int64)
    seg = {}
    for i, u in enumerate(uniq):
        c_, rem = divmod(int(u), NWIN * 2)
        w_, h_ = divmod(rem, 2)
        s, e = starts[i], starts[i + 1]
        counts[c_, w_, h_] = e - s
        seg[(c_, w_, h_)] = (rc[s:e], dec[s:e])

    # uniform block counts
    B = np.maximum(np.ceil(counts / 128.0).astype(np.int64).max(axis=0), 0)
    nb_tot = int(B.sum())
    NT = (nb_tot + K - 1) // K
    pad_blocks = NT * K - nb_tot
    B[NWIN - 1, 1] += pad_blocks  # absorb tile padding into last window half-1
    nb_tot = NT * K

    # block schedule (identical for all cores): list of (win, half),
    # win-major, then re-sorted half-major within each tile so gather runs
    # fragment at most twice per tile.
    blocks = []
    for w_ in range(NWIN):
        for h_ in range(2):
            blocks += [(w_, h_)] * int(B[w_, h_])
    assert len(blocks) == nb_tot
    nb2 = []
    for t in range(NT):
        tb = blocks[t * K:(t + 1) * K]
        nb2 += sorted(tb, key=lambda x: x[1])
    blocks = nb2

    # matmul schedule: (tile, k, win, start, stop). Windows interleave after
    # the half-sort, so start/stop come from global first/last occurrence.
    first_w, last_w = {}, {}
    for b, (w_, h_) in enumerate(blocks):
        first_w.setdefault(w_, b)
        last_w[w_] = b
    sched = []
    for b, (w_, h_) in enumerate(blocks):
        sched.append((b // K, b % K, w_, b == first_w[w_], b == last_w[w_]))
    # psum pool holds 3 window accumulators; verify liveness never exceeds it
    live, mx = set(), 0
    for b, (w_, h_) in enumerate(blocks):
        live.add(w_)
        mx = max(mx, len(live))
        if b == last_w[w_]:
            live.discard(w_)
    assert mx <= 3, f"window liveness {mx} exceeds psum bufs"

    # gather runs: per tile, maximal same-half block runs, capped length
    # >8 blocks (1024 idxs) per dma_gather call crashes the SWDGE path on HW
    RUNCAP = int(os.environ.get("GAT_RUNCAP", "8"))
    runs = []
    for t in range(NT):
        tb = blocks[t * K:(t + 1) * K]
        i = 0
        while i < len(tb):
            j = i
            while j < len(tb) and tb[j][1] == tb[i][1]:
                j += 1
            for c in range(i, j, RUNCAP):
                runs.append((t, c, min(RUNCAP, j - c), tb[i][1]))
            i = j

    # per-core uploads
    idxw = nb_tot * 128 // 16
    idx_cat = np.zeros((NCORE, 128, idxw), np.int16)
    d_e = np.full((NCORE, NT, 128, K), -1.0, np.float32)
    d_eT = np.full((NCORE, NT, K, 128), -1.0, np.float32)
    dstidx = np.full((NCORE, NT, 128, K), -1, np.int32)  # global dst node id
    # global block positions per (win, half) group, in order
    from collections import defaultdict
    gpos = defaultdict(list)
    for gb, (w_, h_) in enumerate(blocks):
        gpos[(w_, h_)].append(gb)

    for c_ in range(NCORE):
        for w_ in range(NWIN):
            for h_ in range(2):
                nb = int(B[w_, h_])
                if nb == 0:
                    continue
                r_arr, de_arr = seg.get((c_, w_, h_), (np.zeros(0, np.int64),) * 2)
                npad = nb * 128 - len(r_arr)
                loc = np.concatenate([r_arr - HALFA * h_, np.full(npad, 0, np.int64)])
                dloc = np.concatenate([de_arr, np.full(npad, -1, np.int64)])
                for b in range(nb):
                    gb = gpos[(w_, h_)][b]
                    t, kk = divmod(gb, K)
                    tok = loc[b * 128:(b + 1) * 128]
                    dl = dloc[b * 128:(b + 1) * 128]
                    dd = dl.astype(np.float32)
                    d_e[c_, t, :, kk] = dd
                    d_eT[c_, t, kk, :] = dd
                    dstidx[c_, t, :, kk] = np.where(
                        dl >= 0, c_ * NLOC + w_ * 128 + dl, -1)
                    # idx wrap: token i at [i%16, gb*8 + i//16], replicated x8
                    wrapped = tok.reshape(8, 16).T.astype(np.int16)  # [16, 8]
                    idx_cat[c_, :, gb * 8:(gb + 1) * 8] = np.tile(wrapped, (8, 1))
    return (NT, sched, runs, idx_cat, d_e.astype(np.float16),
            d_eT.astype(np.float16), dstidx)


def _build_program(NT, sched, runs, idxw):
    nc = bacc.Bacc("TRN2", target_bir_lowering=False, debug=False,
                   num_devices=NCORE, num_swdge_queues=4)
    rec1_d = nc.declare_dram_parameter("rec1", [N, RECE1], F16, isOutput=False)
    # packed per-tile sideband: [d_e (K) | idx (K*8 int16) | ald1pe (K*4, L1)]
    dix_d = nc.declare_dram_parameter("dix", [NT, 128, K * 13], F16,
                                      isOutput=False)
    idx_d = nc.declare_dram_parameter("idx_cat", [128, idxw], I16, isOutput=False)
    de_d = nc.declare_dram_parameter("d_e", [NT, 128, K], F16, isOutput=False)
    deT_d = nc.declare_dram_parameter("d_eT", [NT, K, 128], F16, isOutput=False)
    iota32_d = nc.declare_dram_parameter("iota32", [128], F32, isOutput=False)
    iotajk_d = nc.declare_dram_parameter("iotajk", [JK], F16, isOutput=False)
    w0_d = nc.declare_dram_parameter("w0p", [18, 129], F16, isOutput=False)
    we1_d = nc.declare_dram_parameter("wext1", [129, 135], F16, isOutput=False)
    we2_d = nc.declare_dram_parameter("wext2", [129, 130], F16, isOutput=False)
    out_d = nc.declare_dram_parameter("out", [NLOC, 128], F32, isOutput=True)

    rec2_sa = nc.dram_tensor("rec2_sa", [ROWA * RECE], F16)
    rec2_sb = nc.dram_tensor("rec2_sb", [ROWB * RECE], F16)
    rec3_sa = nc.dram_tensor("rec3_sa", [ROWA * RECE], F16)
    rec3_sb = nc.dram_tensor("rec3_sb", [ROWB * RECE], F16)
    rec2_fa = nc.dram_tensor("rec2_fa", [HALFA, RECE], F16, addr_space="Shared")
    rec2_fb = nc.dram_tensor("rec2_fb", [HALFB, RECE], F16, addr_space="Shared")
    rec3_fa = nc.dram_tensor("rec3_fa", [HALFA, RECE], F16, addr_space="Shared")
    rec3_fb = nc.dram_tensor("rec3_fb", [HALFB, RECE], F16, addr_space="Shared")

    by_tile = {}
    for (t, kk, w_, st, sp) in sched:
        by_tile.setdefault(t, []).append((kk, w_, st, sp))
    runs_by_tile = {}
    for ri, (t, s, nb, hf) in enumerate(runs):
        runs_by_tile.setdefault(t, []).append((ri, s, nb, hf))

    with tile.TileContext(nc) as tc, ExitStack() as ctx:
        RECB = int(os.environ.get("GAT_RECB", "3"))
        SMB = int(os.environ.get("GAT_SMB", "3"))
        recs = ctx.enter_context(tc.tile_pool(name="recs", bufs=RECB))
        pool = ctx.enter_context(tc.tile_pool(name="pool", bufs=2))
        pool3 = ctx.enter_context(tc.tile_pool(name="pool3", bufs=2))
        small3 = ctx.enter_context(tc.tile_pool(name="small3", bufs=SMB))
        singles = ctx.enter_context(tc.tile_pool(name="singles", bufs=1))
        psums = ctx.enter_context(tc.tile_pool(name="psums", bufs=3, space="PSUM"))
        apsums = ctx.enter_context(tc.tile_pool(name="apsums", bufs=2, space="PSUM"))
        npsums = ctx.enter_context(tc.tile_pool(name="npsums", bufs=1, space="PSUM"))
        nptr = ctx.enter_context(tc.tile_pool(name="nptr", bufs=1, space="PSUM"))
        outs = ctx.enter_context(tc.tile_pool(name="outs", bufs=3))

        USE_TS = os.environ.get("GAT_TS", "1") == "1"
        USE_JOUT = os.environ.get("GAT_JOUT", "1") == "1"
        iota_p32 = singles.tile([128, 1], F32)
        nc.sync.dma_start(out=iota_p32[:], in_=bass.AP(
            tensor=iota32_d[:].tensor, offset=0, ap=[[1, 128], [0, 1]]))
        iota_jk = singles.tile([128, JK], F16)
        nc.sync.dma_start(out=iota_jk[:], in_=bass.AP(
            tensor=iotajk_d[:].tensor, offset=0, ap=[[0, 128], [1, JK]]))
        if not USE_TS:
            # per-partition iota as f16: iotajk[j*K] = j
            iota_p16 = singles.tile([128, 1], F16)
            nc.sync.dma_start(out=iota_p16[:], in_=bass.AP(
                tensor=iotajk_d[:].tensor, offset=0, ap=[[K, 128], [0, 1]]))
        if not USE_JOUT:
            # row iota [p, j] = j: iotajk[j*K] = j read with col stride K
            iota_row = singles.tile([128, 128], F16)
            nc.sync.dma_start(out=iota_row[:], in_=bass.AP(
                tensor=iotajk_d[:].tensor, offset=0, ap=[[0, 128], [K, 128]]))
        from concourse.masks import make_identity
        ident = singles.tile([128, 128], F16)
        make_identity(nc, ident[:])
        w0_t = singles.tile([18, 129], F16)
        nc.sync.dma_start(out=w0_t[:], in_=w0_d[:])
        we1_t = singles.tile([128, 135], F16)
        nc.sync.dma_start(out=we1_t[:], in_=we1_d[0:128, :])
        we1b_t = singles.tile([1, 135], F16)
        nc.sync.dma_start(out=we1b_t[:], in_=we1_d[128:129, :])
        we2_t = singles.tile([128, 130], F16)
        nc.sync.dma_start(out=we2_t[:], in_=we2_d[0:128, :])
        we2b_t = singles.tile([1, 130], F16)
        nc.sync.dma_start(out=we2b_t[:], in_=we2_d[128:129, :])

        # al_d stages for layers 2/3: [p, w*H4] f16 ; node (w,p) at col w*H4
        # (heads padded to 4 so the whole logits pipeline is (k,h4)-wide).
        # Layer 1's al_d is host-precomputed per edge (ald1pe_d).
        H4 = 4
        ald2_t = singles.tile([128, NWIN * H4], F16)
        nc.vector.memset(ald2_t[:], 0.0)
        ald3_t = singles.tile([128, NWIN], F16)
        nc.vector.memset(ald3_t[:], 0.0)

        gather_ctr = [0]  # DMASW sems pair queues by emission order (mod 8/4)

        def edge_phase(layer):
            Hw = 4 if layer < 3 else 1   # padded head width of the w pipeline
            rece = RECE1 if layer == 1 else RECE
            # rhs column width per block (f,h4-interleaved for L1/L2)
            rhsw = 28 if layer == 1 else (176 if layer == 2 else 129)
            ald_t = (None, ald2_t, ald3_t)[layer - 1]
            psum_win = {}
            for t in range(NT):
                rec_t = recs.tile([128, K * rece], F16, tag="rec")
                de_t = small3.tile([128, K], F16, tag="de")
                oh1_t = pool3.tile([128, JK], F16, tag="oh1")
                lg_t = small3.tile([128, K * Hw], F32, tag="lg")
                tmp_t = small3.tile([128, K * Hw], F32, tag="tmp")
                w4_t = small3.tile([128, K * Hw], F16, tag="w")
                idx_t = small3.tile([128, K * 8], I16, tag="idx")
                if layer != 3:
                    rhs_t = pool3.tile([128, K * rhsw], F16, tag="rhs")

                nc.sync.dma_start(out=de_t[:], in_=de_d[t])
                nc.sync.dma_start(out=idx_t[:], in_=idx_d[:, t * K * 8:(t + 1) * K * 8])
                if layer == 1:
                    ald1pe_t = small3.tile([128, K * 4], F16, tag="ald1pe")
                    nc.sync.dma_start(out=ald1pe_t[:], in_=ald1pe_d[t])
                else:
                    deT_t = pool.tile([128, JK], F16, tag="deT")
                    nc.sync.dma_start(out=deT_t[:], in_=bass.AP(
                        tensor=deT_d[:].tensor, offset=t * JK,
                        ap=[[0, 128], [1, JK]]))

                for (ri, s, nb, hf) in runs_by_tile[t]:
                    n_idx = nb * 128
                    if layer == 1:
                        in_ap = rec1_d[HALFA:, :] if hf else rec1_d[0:HALFA, :]
                    elif layer == 2:
                        in_ap = rec2_fb[:] if hf else rec2_fa[:]
                    else:
                        in_ap = rec3_fb[:] if hf else rec3_fa[:]
                    base = rec_t[:]
                    out_ap = bass.AP(
                        tensor=base.tensor, offset=base.offset + s * rece,
                        ap=[list(base.ap[0]), [rece, nb], [1, rece]])
                    nc.gpsimd.dma_gather(
                        out_ap=out_ap, in_ap=in_ap,
                        idxs_ap=idx_t[:, s * 8:(s + nb) * 8],
                        num_idxs=n_idx, num_idxs_reg=n_idx, elem_size=rece,
                        queue_num=gather_ctr[0] % 4)
                    gather_ctr[0] += 1

                # one-hot, edge partitions: oh1[e, j*K+k] = (de[e,k] == j)
                # (or [e, k*128+j] when USE_JOUT is off)
                if USE_JOUT:
                    nc.vector.tensor_tensor(
                        out=oh1_t[:],
                        in0=ap_of(de_t, 0, [[0, 128], [1, K]]),
                        in1=iota_jk[:],
                        op=mybir.AluOpType.is_equal)

                    def oh1_lhsT(kk):
                        return ap_of(oh1_t, kk, [[K, 128]])
                else:
                    nc.vector.tensor_tensor(
                        out=oh1_t[:],
                        in0=ap_of(de_t, 0, [[1, K], [0, 128]]),
                        in1=ap_of(iota_row, 0, [[0, K], [1, 128]]),
                        op=mybir.AluOpType.is_equal)

                    def oh1_lhsT(kk):
                        return oh1_t[:, kk * 128:(kk + 1) * 128]
                if layer > 1:
                    # one-hot, dst partitions: oh2[p, k*128+e] = (de[k,e] == p)
                    oh2_t = deT_t
                    if USE_TS:
                        nc.vector.tensor_scalar(
                            deT_t[:], deT_t[:], iota_p32[:], None,
                            mybir.AluOpType.is_equal)
                    else:
                        nc.vector.tensor_tensor(
                            out=deT_t[:], in0=deT_t[:],
                            in1=ap_of(iota_p16, 0, [[0, JK]]),
                            op=mybir.AluOpType.is_equal)

                    ald_ps = apsums.tile([128, K * Hw], F32, tag="aldps",
                                         name="ald_ps")
                    for (kk, w_, st, sp) in by_tile[t]:
                        nc.tensor.matmul(
                            out=ald_ps[:, kk * Hw:(kk + 1) * Hw],
                            lhsT=oh2_t[:, kk * 128:(kk + 1) * 128],
                            rhs=ald_t[:, w_ * Hw:(w_ + 1) * Hw],
                            start=True, stop=True)
                    ald_in = ald_ps[:]
                else:
                    ald_in = ald1pe_t[:]

                if layer == 1:
                    als_ap = ap_of(rec_t[:].bitcast(F32), 14, [[RECE1 // 2, K], [1, Hw]])
                elif layer == 2:
                    als_ap = ap_of(rec_t[:].bitcast(F32), 88, [[RECE // 2, K], [1, Hw]])
                else:
                    als_ap = ap_of(rec_t[:].bitcast(F32), 65, [[RECE // 2, K], [1, Hw]])
                nc.vector.tensor_add(out=lg_t[:], in0=als_ap, in1=ald_in)
                nc.vector.tensor_scalar_mul(out=tmp_t[:], in0=lg_t[:], scalar1=NEG)
                nc.vector.tensor_max(out=lg_t[:], in0=lg_t[:], in1=tmp_t[:])
                nc.scalar.activation(out=w4_t[:], in_=lg_t[:],
                                     func=mybir.ActivationFunctionType.Exp)

                if layer == 1:
                    # rhs[e, (k,f,h4)] = rec[e,(k,f,h4)] * w4[e,(k,h4)]
                    rhs_in0 = ap_of(rec_t, 0, [[RECE1, K], [1, 28]])
                    rhs_in1 = ap_of(w4_t, 0, [[4, K], [0, 7], [1, 4]])
                    nc.vector.tensor_tensor(out=rhs_t[:], in0=rhs_in0,
                                            in1=rhs_in1, op=mybir.AluOpType.mult)
                elif layer == 2:
                    rhs_in0 = ap_of(rec_t, 0, [[RECE, K], [1, 176]])
                    rhs_in1 = ap_of(w4_t, 0, [[4, K], [0, 44], [1, 4]])
                    nc.vector.tensor_tensor(out=rhs_t[:], in0=rhs_in0,
                                            in1=rhs_in1, op=mybir.AluOpType.mult)
                else:
                    # fold w into the one-hot; raw record is the matmul rhs
                    w_bcast = (ap_of(w4_t, 0, [[0, 128], [1, K]]) if USE_JOUT
                               else ap_of(w4_t, 0, [[1, K], [0, 128]]))
                    nc.vector.tensor_tensor(
                        out=oh1_t[:], in0=oh1_t[:], in1=w_bcast,
                        op=mybir.AluOpType.mult)

                for (kk, w_, st, sp) in by_tile[t]:
                    if st:
                        psum_win[w_] = psums.tile([128, rhsw], F32,
                                                  tag="agg", name="agg_ps")
                    if layer != 3:
                        rhs_ap = rhs_t[:, kk * rhsw:(kk + 1) * rhsw]
                    else:
                        rhs_ap = ap_of(rec_t, kk * RECE, [[1, 129]])
                    nc.tensor.matmul(
                        out=psum_win[w_][:],
                        lhsT=oh1_lhsT(kk),
                        rhs=rhs_ap,
                        start=st, stop=sp)
                    if sp:
                        finalize(layer, w_, psum_win.pop(w_))

        def finalize(layer, w_, ps):
            rows = LASTW if w_ == NWIN - 1 else 128
            if layer == 1:
                recip = outs.tile([128, H], F32, tag="recip1")
                nc.vector.reciprocal(out=recip[:], in_=ap_of(ps, 24, [[1, H]]))
                xn_t = outs.tile([128, 18], F16, tag="xn")
                nc.vector.tensor_tensor(
                    out=ap_of(xn_t, 0, [[6, H], [1, 6]]),
                    in0=ap_of(ps, 0, [[1, H], [4, 6]]),
                    in1=ap_of(recip, 0, [[1, H], [0, 6]]),
                    op=mybir.AluOpType.mult)
                xT_ps = nptr.tile([18, 128], F16, tag="xT", name="xT_ps")
                nc.tensor.transpose(out=xT_ps[:], in_=xn_t[:], identity=ident[:])
                xT_t = outs.tile([18, 128], F16, tag="xTs")
                nc.vector.tensor_copy(out=xT_t[:], in_=xT_ps[:])
                g_ps = npsums.tile([128, 129], F32, tag="npA", name="g1_ps")
                nc.tensor.matmul(out=g_ps[:], lhsT=xT_t[:], rhs=w0_t[:],
                                 start=True, stop=True)
                node_phase(1, w_, g_ps, rows)
            elif layer == 2:
                recip = outs.tile([128, H], F32, tag="recip2")
                nc.vector.reciprocal(out=recip[:], in_=ap_of(ps, 172, [[1, H]]))
                g_t = outs.tile([128, 129], F32, tag="g2pre")
                nc.vector.tensor_tensor(
                    out=ap_of(g_t, 0, [[F, H], [1, F]]),
                    in0=ap_of(ps, 0, [[1, H], [4, F]]),
                    in1=ap_of(recip, 0, [[1, H], [0, F]]),
                    op=mybir.AluOpType.mult)
                node_phase(2, w_, g_t, rows)
            else:
                recip = outs.tile([128, 1], F32, tag="recip3")
                nc.vector.reciprocal(out=recip[:], in_=ps[:, 128:129])
                o_t = outs.tile([128, 128], F32, tag="ofin")
                nc.vector.tensor_tensor(
                    out=o_t[:], in0=ps[:, 0:128],
                    in1=ap_of(recip, 0, [[0, 128]]),
                    op=mybir.AluOpType.mult)
                nc.sync.dma_start(out=out_d[w_ * 128:w_ * 128 + rows, :],
                                  in_=o_t[0:rows, :])

        def node_phase(layer, w_, g_in, rows):
            # g_in: layer-1 -> psum [128,129] f32 pre-activation; layer-2 -> sbuf f32
            tmp_t = outs.tile([128, 129], F32, tag="nltmp")
            gl_t = outs.tile([128, 129], F16, tag="nlgl")
            nc.vector.tensor_scalar_mul(out=tmp_t[:], in0=g_in[:, 0:129], scalar1=NEG)
            nc.vector.tensor_max(out=gl_t[:], in0=g_in[:, 0:129], in1=tmp_t[:])
            t01_ps = nptr.tile([128, 256], F16, tag="t01", name="t01_ps")
            nc.tensor.transpose(out=t01_ps[:, 0:128], in_=gl_t[:, 0:128],
                                identity=ident[:])
            nc.tensor.transpose(out=t01_ps[0:1, 128:256], in_=gl_t[:, 128:129],
                                identity=ident[:])
            gT0 = outs.tile([128, 128], F16, tag="gT0")
            gT1 = outs.tile([1, 128], F16, tag="gT1")
            nc.vector.tensor_copy(out=gT0[:], in_=t01_ps[:, 0:128])
            nc.vector.tensor_copy(out=gT1[:], in_=t01_ps[0:1, 128:256])
            wa, wb = (we1_t, we1b_t) if layer == 1 else (we2_t, we2b_t)
            wcols = 135 if layer == 1 else 130
            h_ps = npsums.tile([128, wcols], F32, tag="npA", name="h_ps")
            nc.tensor.matmul(out=h_ps[:], lhsT=gT0[:], rhs=wa[:], start=True, stop=False)
            nc.tensor.matmul(out=h_ps[:], lhsT=gT1[:], rhs=wb[:], start=False, stop=True)
            rec_t = outs.tile([128, RECE], F16, tag="recslice")
            nc.vector.memset(rec_t[:], 1.0)
            if layer == 1:
                # L2 record: (f,h4) interleave of the 129 feats; ones at 172:176
                nc.vector.tensor_copy(
                    out=ap_of(rec_t, 0, [[4, F], [1, H]]),
                    in_=ap_of(h_ps, 0, [[1, F], [F, H]]))
                nc.vector.tensor_copy(
                    out=ap_of(rec_t[:].bitcast(F32), 88, [[1, H]]),
                    in_=h_ps[:, 129:132])
                nc.vector.tensor_copy(out=ald2_t[0:rows, w_ * 4:w_ * 4 + H],
                                      in_=h_ps[0:rows, 132:135])
                sa, sb = rec2_sa, rec2_sb
            else:
                nc.vector.tensor_copy(out=rec_t[:, 0:128], in_=h_ps[:, 0:128])
                nc.vector.tensor_copy(
                    out=ap_of(rec_t[:].bitcast(F32), 65, [[1, 1]]),
                    in_=h_ps[:, 128:129])
                nc.vector.tensor_copy(out=ald3_t[0:rows, w_:w_ + 1],
                                      in_=h_ps[0:rows, 129:130])
                sa, sb = rec3_sa, rec3_sb
            if w_ < WINA:
                dst_dram, off = sa, w_ * 128 * RECE
            else:
                dst_dram, off = sb, (w_ * 128 - ROWA) * RECE
            nc.sync.dma_start(
                out=bass.AP(tensor=dst_dram[:].tensor, offset=off,
                            ap=[[RECE, rows], [1, RECE]]),
                in_=rec_t[0:rows, :])

        def gather_halves(sa, sb, fa, fb):
            nc.gpsimd.collective_compute(
                "AllGather", mybir.AluOpType.bypass,
                replica_groups=[list(range(NCORE))],
                ins=[sa[:]], outs=[fa[:].rearrange("a b -> (a b)")])
            nc.gpsimd.collective_compute(
                "AllGather", mybir.AluOpType.bypass,
                replica_groups=[list(range(NCORE))],
                ins=[sb[:]], outs=[fb[:].rearrange("a b -> (a b)")])

        edge_phase(1)
        gather_halves(rec2_sa, rec2_sb, rec2_fa, rec2_fb)
        edge_phase(2)
        gather_halves(rec3_sa, rec3_sb, rec3_fa, rec3_fb)
        edge_phase(3)

    nc.compile()
    return nc


_CACHE = {}


def run(inputs, trace=False):
    x = np.asarray(inputs["x"], np.float32)
    ei = np.asarray(inputs["edge_index"]).astype(np.int64)
    W0 = np.asarray(inputs["W0"], np.float32)
    a_src0 = np.asarray(inputs["a_src0"], np.float32)
    a_dst0 = np.asarray(inputs["a_dst0"], np.float32)
    b0 = np.asarray(inputs["b0"], np.float32)
    W1 = np.asarray(inputs["W1"], np.float32)
    a_src1 = np.asarray(inputs["a_src1"], np.float32)
    a_dst1 = np.asarray(inputs["a_dst1"], np.float32)
    b1 = np.asarray(inputs["b1"], np.float32)
    W2 = np.asarray(inputs["W2"], np.float32)
    a_src2 = np.asarray(inputs["a_src2"], np.float32)
    a_dst2 = np.asarray(inputs["a_dst2"], np.float32)
    b2 = np.asarray(inputs["b2"], np.float32)
    assert np.all(b0 == 0) and np.all(b1 == 0), "nonzero hidden biases unsupported"

    loops = np.arange(N, dtype=np.int64)
    src = np.concatenate([ei[0], loops])
    dst = np.concatenate([ei[1], loops])

    skey = hash((src.tobytes(), dst.tobytes()))
    if "struct" not in _CACHE or _CACHE.get("skey") != skey:
        struct = _build_structure(src, dst)
        _CACHE.update(skey=skey, struct=struct)
        _CACHE.pop("nc", None)
    NT, sched, runs, idx_cat, d_e, d_eT, dstidx = _CACHE["struct"]
    if "nc" not in _CACHE:
        _CACHE["nc"] = _build_program(NT, sched, runs, idx_cat.shape[2])
    nc = _CACHE["nc"]

    # host precompute: layer-1 tables, extended weight matrices
    c_s0 = np.stack([W0[:, h * F:(h + 1) * F] @ a_src0[h] for h in range(H)], 1)
    c_d0 = np.stack([W0[:, h * F:(h + 1) * F] @ a_dst0[h] for h in range(H)], 1)
    al_s1 = x @ c_s0
    al_d1 = x @ c_d0
    # L1 record, (f,h4) layout: cols f*4+h = x_f (f<6) / 1.0 (f=6); als f32@14:17
    rec1 = np.zeros((N, RECE1), np.float16)
    xf = x.astype(np.float16)
    for f_ in range(6):
        for h_ in range(4):
            rec1[:, f_ * 4 + h_] = xf[:, f_]
    rec1[:, 24:28] = 1.0
    rec1[:, 28:34] = al_s1.astype(np.float32).view(np.uint16).reshape(N, 6).view(np.float16)
    # remap to table-row order
    perm = _remap_rows(np.arange(N, dtype=np.int64))
    rec1_tbl = np.zeros_like(rec1)
    rec1_tbl[perm] = rec1

    def wext(W, a_s, a_d, heads, f):
        cs = np.stack([W[:, h * f:(h + 1) * f] @ a_s[h] for h in range(heads)], 1)
        cd = np.stack([W[:, h * f:(h + 1) * f] @ a_d[h] for h in range(heads)], 1)
        return np.concatenate([W, cs, cd], axis=1).astype(np.float16)

    we1 = wext(W1, a_src1, a_dst1, 3, F)          # [129, 135]
    we2 = wext(W2, a_src2, a_dst2, 1, 128)        # [129, 130]
    w0p = np.zeros((18, 129), np.float16)         # block-diag [3x(6,43)]
    for h in range(H):
        w0p[6 * h:6 * h + 6, F * h:F * (h + 1)] = W0[:, F * h:F * (h + 1)].astype(np.float16)
    iota32 = np.arange(128, dtype=np.float32)
    iotajk = (np.arange(JK) // K).astype(np.float16)

    in_maps = []
    for c in range(NCORE):
        dsti = dstidx[c]                       # [NT, 128, K] int32
        vals = al_d1[np.maximum(dsti, 0)]      # [NT, 128, K, H] f32
        vals[dsti < 0] = 0.0
        ald1pe = np.zeros(dsti.shape + (4,), np.float16)
        ald1pe[..., :H] = vals.astype(np.float16)
        ald1pe = ald1pe.reshape(NT, 128, K * 4)
        in_maps.append(dict(
            rec1=rec1_tbl, ald1pe=ald1pe, idx_cat=idx_cat[c], d_e=d_e[c],
            d_eT=d_eT[c], iota32=iota32, iotajk=iotajk, w0p=w0p,
            wext1=we1, wext2=we2))

    res = run_bass_kernel_spmd(nc, in_maps, list(range(NCORE)), trace=trace)
    out = np.concatenate([res.results[c]["out"] for c in range(NCORE)], axis=0)
    out = out + b2[None, :]
    return out.astype(np.float32), res


def kernel(**inputs) -> np.ndarray:
    out, _ = run(inputs, trace=False)
    return out


# revision 44
# speedup vs baseline: 1.0678x; 1.0678x over previous
"""3-layer GAT encoder on 8 trn2 NeuronCores (Bass/Tile).

Strategy: edge-parallel sharding by destination node block (core k owns dst
nodes [k*6250, (k+1)*6250)), so all segment ops are core-local. Per layer the
aggregation is factored as out[n,h] = (sum_e w_e * hfeat[src_e]) / (sum_e w_e)
with w_e = exp(leaky_relu(al_s[src] + al_d[dst])). Per-edge work is done in
128-edge blocks: src features come from a dma_gather of 512-byte fp16 node
records (two table halves for int16 indices), al_d[dst] is broadcast via a
onehot matmul, and the segment sum is an edge-orientation onehot matmul
accumulated in PSUM per 128-dst-node window.

Perf notes vs the first version:
 - one-hots are built in DVE 2x/4x perf modes: oh2 (dst-part orientation) via
   tensor_scalar with a per-partition f32 iota scalar (single-src 4x); oh1
   (edge-part orientation) in (j-outer, k-inner) column order against a
   materialized iota pattern so both tensor_tensor operands are unit-stride.
 - records store features interleaved (f, h4) with heads padded to 4 so the
   per-edge weight multiply has both operands unit-stride (2x mode). Layer 3
   (1 head) instead folds w into the one-hot and streams the raw record as
   the matmul rhs.
 - the node table is split into two half tensors; each half's AllGather fires
   as soon as its windows finalize, overlapping the collective with the edge
   phase tail and the next layer's start. Node->table-row order is remapped
   (half-major, then rank-major) so AllGather's rank-major concat lands rows
   exactly where the gather indices expect them.
"""
import os
import numpy as np
from contextlib import ExitStack

import concourse.bass as bass
import concourse.bacc as bacc
import concourse.tile as tile
from concourse import mybir
from concourse.bass_utils import run_bass_kernel_spmd

F16 = mybir.dt.float16
F32 = mybir.dt.float32
I16 = mybir.dt.int16

N = 50000
NCORE = 8
NLOC = N // NCORE            # 6250
NWIN = (NLOC + 127) // 128   # 49
LASTW = NLOC - 128 * (NWIN - 1)  # 106
WINA = 25                    # windows in half A
ROWA = WINA * 128            # 3200 local rows in half A
ROWB = NLOC - ROWA           # 3050 local rows in half B
HALFA = NCORE * ROWA         # 25600 table rows in half A
HALFB = NCORE * ROWB         # 24400
H, F = 3, 43
NEG = 0.2
RECE1 = 128                  # f16 record L1: [(x6+1,h4)=28, pad, al_s f32@14:17]
RECE = 256                   # f16 record L2/L3 (512B)
K = 48                       # blocks per tile
JK = K * 128                 # one-hot cols per tile


def ap_of(t, offset_elems, dims):
    base = t if isinstance(t, bass.AP) else t[:]
    return bass.AP(tensor=base.tensor, offset=base.offset + offset_elems,
                   ap=[list(base.ap[0])] + [list(d) for d in dims])


def _remap_rows(src):
    """Global node id -> table row (half-major, rank-major, local)."""
    c = src // NLOC
    r = src - c * NLOC
    return np.where(r < ROWA, c * ROWA + r, HALFA + c * ROWB + (r - ROWA))


def _build_structure(src, dst):
    """Host: shard edges by dst core / 128-window / src half, uniform block
    structure across cores. Returns per-core upload arrays + schedule."""
    core = dst // NLOC
    dst_loc = dst - core * NLOC
    win = dst_loc // 128
    de = dst_loc % 128
    row = _remap_rows(src)
    half = (row >= HALFA).astype(np.int64)

    # bucket edges per (core, win, half)
    order = np.lexsort((half, win, core))
    rc, wc, hc, dec = row[order], win[order], half[order], de[order]
    key = ((core[order] * NWIN + wc) * 2 + hc)
    uniq, starts = np.unique(key, return_index=True)
    starts = list(starts) + [len(key)]
    counts = np.zeros((NCORE, NWIN, 2), np.int64)
    seg = {}
    for i, u in enumerate(uniq):
        c_, rem = divmod(int(u), NWIN * 2)
        w_, h_ = divmod(rem, 2)
        s, e = starts[i], starts[i + 1]
        counts[c_, w_, h_] = e - s
        seg[(c_, w_, h_)] = (rc[s:e], dec[s:e])

    # uniform block counts
    B = np.maximum(np.ceil(counts / 128.0).astype(np.int64).max(axis=0), 0)
    nb_tot = int(B.sum())
    NT = (nb_tot + K - 1) // K
    pad_blocks = NT * K - nb_tot
    B[NWIN - 1, 1] += pad_blocks  # absorb tile padding into last window half-1
    nb_tot = NT * K

    # block schedule (identical for all cores): list of (win, half)
    blocks = []
    for w_ in range(NWIN):
        for h_ in range(2):
            blocks += [(w_, h_)] * int(B[w_, h_])
    assert len(blocks) == nb_tot

    # matmul schedule: (tile, k, win, start, stop)
    sched = []
    prev_w = -1
    for b, (w_, h_) in enumerate(blocks):
        st = w_ != prev_w
        sp = (b == nb_tot - 1) or (blocks[b + 1][0] != w_)
        sched.append((b // K, b % K, w_, st, sp))
        prev_w = w_

    # gather runs: per tile, maximal same-half block runs, capped length
    # >8 blocks (1024 idxs) per dma_gather call crashes the SWDGE path on HW
    RUNCAP = int(os.environ.get("GAT_RUNCAP", "8"))
    runs = []
    for t in range(NT):
        tb = blocks[t * K:(t + 1) * K]
        i = 0
        while i < len(tb):
            j = i
            while j < len(tb) and tb[j][1] == tb[i][1]:
                j += 1
            for c in range(i, j, RUNCAP):
                runs.append((t, c, min(RUNCAP, j - c), tb[i][1]))
            i = j

    # per-core uploads
    idxw = nb_tot * 128 // 16
    idx_cat = np.zeros((NCORE, 128, idxw), np.int16)
    d_e = np.full((NCORE, NT, 128, K), -1.0, np.float32)
    d_eT = np.full((NCORE, NT, K, 128), -1.0, np.float32)
    dstidx = np.full((NCORE, NT, 128, K), -1, np.int32)  # global dst node id
    # global block positions per (win, half) group, in order
    from collections import defaultdict
    gpos = defaultdict(list)
    for gb, (w_, h_) in enumerate(blocks):
        gpos[(w_, h_)].append(gb)

    for c_ in range(NCORE):
        for w_ in range(NWIN):
            for h_ in range(2):
                nb = int(B[w_, h_])
                if nb == 0:
                    continue
                r_arr, de_arr = seg.get((c_, w_, h_), (np.zeros(0, np.int64),) * 2)
                npad = nb * 128 - len(r_arr)
                loc = np.concatenate([r_arr - HALFA * h_, np.full(npad, 0, np.int64)])
                dloc = np.concatenate([de_arr, np.full(npad, -1, np.int64)])
                for b in range(nb):
                    gb = gpos[(w_, h_)][b]
                    t, kk = divmod(gb, K)
                    tok = loc[b * 128:(b + 1) * 128]
                    dl = dloc[b * 128:(b + 1) * 128]
                    dd = dl.astype(np.float32)
                    d_e[c_, t, :, kk] = dd
                    d_eT[c_, t, kk, :] = dd
                    dstidx[c_, t, :, kk] = np.where(
                        dl >= 0, c_ * NLOC + w_ * 128 + dl, -1)
                    # idx wrap: token i at [i%16, gb*8 + i//16], replicated x8
                    wrapped = tok.reshape(8, 16).T.astype(np.int16)  # [16, 8]
                    idx_cat[c_, :, gb * 8:(gb + 1) * 8] = np.tile(wrapped, (8, 1))
    return (NT, sched, runs, idx_cat, d_e.astype(np.float16),
            d_eT.astype(np.float16), dstidx)


def _build_program(NT, sched, runs, idxw):
    nc = bacc.Bacc("TRN2", target_bir_lowering=False, debug=False,
                   num_devices=NCORE, num_swdge_queues=4)
    rec1_d = nc.declare_dram_parameter("rec1", [N, RECE1], F16, isOutput=False)
    # packed per-tile sideband: [d_e (K) | idx (K*8 int16) | ald1pe (K*4, L1)]
    dix_d = nc.declare_dram_parameter("dix", [NT, 128, K * 13], F16,
                                      isOutput=False)
    deT_d = nc.declare_dram_parameter("d_eT", [NT, K, 128], F16, isOutput=False)
    iota32_d = nc.declare_dram_parameter("iota32", [128], F32, isOutput=False)
    iotajk_d = nc.declare_dram_parameter("iotajk", [JK], F16, isOutput=False)
    w0_d = nc.declare_dram_parameter("w0p", [18, 129], F16, isOutput=False)
    we1_d = nc.declare_dram_parameter("wext1", [129, 135], F16, isOutput=False)
    we2_d = nc.declare_dram_parameter("wext2", [129, 130], F16, isOutput=False)
    out_d = nc.declare_dram_parameter("out", [NLOC, 128], F32, isOutput=True)

    rec2_sa = nc.dram_tensor("rec2_sa", [ROWA * RECE], F16)
    rec2_sb = nc.dram_tensor("rec2_sb", [ROWB * RECE], F16)
    rec3_sa = nc.dram_tensor("rec3_sa", [ROWA * RECE], F16)
    rec3_sb = nc.dram_tensor("rec3_sb", [ROWB * RECE], F16)
    rec2_fa = nc.dram_tensor("rec2_fa", [HALFA, RECE], F16, addr_space="Shared")
    rec2_fb = nc.dram_tensor("rec2_fb", [HALFB, RECE], F16, addr_space="Shared")
    rec3_fa = nc.dram_tensor("rec3_fa", [HALFA, RECE], F16, addr_space="Shared")
    rec3_fb = nc.dram_tensor("rec3_fb", [HALFB, RECE], F16, addr_space="Shared")

    by_tile = {}
    for (t, kk, w_, st, sp) in sched:
        by_tile.setdefault(t, []).append((kk, w_, st, sp))
    runs_by_tile = {}
    for ri, (t, s, nb, hf) in enumerate(runs):
        runs_by_tile.setdefault(t, []).append((ri, s, nb, hf))

    with tile.TileContext(nc) as tc, ExitStack() as ctx:
        RECB = int(os.environ.get("GAT_RECB", "3"))
        SMB = int(os.environ.get("GAT_SMB", "3"))
        recs = ctx.enter_context(tc.tile_pool(name="recs", bufs=RECB))
        pool = ctx.enter_context(tc.tile_pool(name="pool", bufs=2))
        pool3 = ctx.enter_context(tc.tile_pool(name="pool3", bufs=2))
        small3 = ctx.enter_context(tc.tile_pool(name="small3", bufs=SMB))
        singles = ctx.enter_context(tc.tile_pool(name="singles", bufs=1))
        psums = ctx.enter_context(tc.tile_pool(name="psums", bufs=2, space="PSUM"))
        apsums = ctx.enter_context(tc.tile_pool(name="apsums", bufs=2, space="PSUM"))
        npsums = ctx.enter_context(tc.tile_pool(name="npsums", bufs=2, space="PSUM"))
        nptr = ctx.enter_context(tc.tile_pool(name="nptr", bufs=1, space="PSUM"))
        outs = ctx.enter_context(tc.tile_pool(name="outs", bufs=3))

        USE_TS = os.environ.get("GAT_TS", "1") == "1"
        USE_JOUT = os.environ.get("GAT_JOUT", "1") == "1"
        iota_p32 = singles.tile([128, 1], F32)
        nc.sync.dma_start(out=iota_p32[:], in_=bass.AP(
            tensor=iota32_d[:].tensor, offset=0, ap=[[1, 128], [0, 1]]))
        iota_jk = singles.tile([128, JK], F16)
        nc.sync.dma_start(out=iota_jk[:], in_=bass.AP(
            tensor=iotajk_d[:].tensor, offset=0, ap=[[0, 128], [1, JK]]))
        if not USE_TS:
            # per-partition iota as f16: iotajk[j*K] = j
            iota_p16 = singles.tile([128, 1], F16)
            nc.sync.dma_start(out=iota_p16[:], in_=bass.AP(
                tensor=iotajk_d[:].tensor, offset=0, ap=[[K, 128], [0, 1]]))
        if not USE_JOUT:
            # row iota [p, j] = j: iotajk[j*K] = j read with col stride K
            iota_row = singles.tile([128, 128], F16)
            nc.sync.dma_start(out=iota_row[:], in_=bass.AP(
                tensor=iotajk_d[:].tensor, offset=0, ap=[[0, 128], [K, 128]]))
        from concourse.masks import make_identity
        ident = singles.tile([128, 128], F16)
        make_identity(nc, ident[:])
        w0_t = singles.tile([18, 129], F16)
        nc.sync.dma_start(out=w0_t[:], in_=w0_d[:])
        we1_t = singles.tile([128, 135], F16)
        nc.sync.dma_start(out=we1_t[:], in_=we1_d[0:128, :])
        we1b_t = singles.tile([1, 135], F16)
        nc.sync.dma_start(out=we1b_t[:], in_=we1_d[128:129, :])
        we2_t = singles.tile([128, 130], F16)
        nc.sync.dma_start(out=we2_t[:], in_=we2_d[0:128, :])
        we2b_t = singles.tile([1, 130], F16)
        nc.sync.dma_start(out=we2b_t[:], in_=we2_d[128:129, :])

        # al_d stages for layers 2/3: [p, w*H4] f16 ; node (w,p) at col w*H4
        # (heads padded to 4 so the whole logits pipeline is (k,h4)-wide).
        # Layer 1's al_d is host-precomputed per edge (ald1pe_d).
        H4 = 4
        ald2_t = singles.tile([128, NWIN * H4], F16)
        nc.vector.memset(ald2_t[:], 0.0)
        ald3_t = singles.tile([128, NWIN], F16)
        nc.vector.memset(ald3_t[:], 0.0)

        gather_ctr = [0]  # DMASW sems pair queues by emission order (mod 8/4)

        def edge_phase(layer):
            Hw = 4 if layer < 3 else 1   # padded head width of the w pipeline
            rece = RECE1 if layer == 1 else RECE
            # rhs column width per block (f,h4-interleaved for L1/L2)
            rhsw = 28 if layer == 1 else (176 if layer == 2 else 129)
            ald_t = (None, ald2_t, ald3_t)[layer - 1]
            psum_win = {}
            dixw = K * 13 if layer == 1 else K * 9
            for t in range(NT):
                rec_t = recs.tile([128, K * rece], F16, tag="rec")
                dix_t = small3.tile([128, dixw], F16, tag="dix")
                oh1_t = pool3.tile([128, JK], F16, tag="oh1")
                lg_t = small3.tile([128, K * Hw], F32, tag="lg")
                tmp_t = small3.tile([128, K * Hw], F32, tag="tmp")
                w4_t = small3.tile([128, K * Hw], F16, tag="w")
                if layer != 3:
                    rhs_t = pool3.tile([128, K * rhsw], F16, tag="rhs")

                # packed sideband: de [0:K], idx [K:K*9], ald1pe [K*9:K*13]
                nc.sync.dma_start(out=dix_t[:], in_=bass.AP(
                    tensor=dix_d[:].tensor, offset=t * 128 * K * 13,
                    ap=[[K * 13, 128], [1, dixw]]))
                de_t = dix_t
                if layer > 1:
                    deT_t = pool.tile([128, JK], F16, tag="deT")
                    nc.sync.dma_start(out=deT_t[:], in_=bass.AP(
                        tensor=deT_d[:].tensor, offset=t * JK,
                        ap=[[0, 128], [1, JK]]))

                for (ri, s, nb, hf) in runs_by_tile[t]:
                    n_idx = nb * 128
                    if layer == 1:
                        in_ap = rec1_d[HALFA:, :] if hf else rec1_d[0:HALFA, :]
                    elif layer == 2:
                        in_ap = rec2_fb[:] if hf else rec2_fa[:]
                    else:
                        in_ap = rec3_fb[:] if hf else rec3_fa[:]
                    base = rec_t[:]
                    out_ap = bass.AP(
                        tensor=base.tensor, offset=base.offset + s * rece,
                        ap=[list(base.ap[0]), [rece, nb], [1, rece]])
                    nc.gpsimd.dma_gather(
                        out_ap=out_ap, in_ap=in_ap,
                        idxs_ap=ap_of(dix_t[:].bitcast(I16), K + s * 8,
                                      [[1, nb * 8]]),
                        num_idxs=n_idx, num_idxs_reg=n_idx, elem_size=rece,
                        queue_num=gather_ctr[0] % 4)
                    gather_ctr[0] += 1

                # one-hot, edge partitions: oh1[e, j*K+k] = (de[e,k] == j)
                # (or [e, k*128+j] when USE_JOUT is off)
                if USE_JOUT:
                    nc.vector.tensor_tensor(
                        out=oh1_t[:],
                        in0=ap_of(de_t, 0, [[0, 128], [1, K]]),
                        in1=iota_jk[:],
                        op=mybir.AluOpType.is_equal)

                    def oh1_lhsT(kk):
                        return ap_of(oh1_t, kk, [[K, 128]])
                else:
                    nc.vector.tensor_tensor(
                        out=oh1_t[:],
                        in0=ap_of(de_t, 0, [[1, K], [0, 128]]),
                        in1=ap_of(iota_row, 0, [[0, K], [1, 128]]),
                        op=mybir.AluOpType.is_equal)

                    def oh1_lhsT(kk):
                        return oh1_t[:, kk * 128:(kk + 1) * 128]
                if layer > 1:
                    # one-hot, dst partitions: oh2[p, k*128+e] = (de[k,e] == p)
                    oh2_t = deT_t
                    if USE_TS:
                        nc.vector.tensor_scalar(
                            deT_t[:], deT_t[:], iota_p32[:], None,
                            mybir.AluOpType.is_equal)
                    else:
                        nc.vector.tensor_tensor(
                            out=deT_t[:], in0=deT_t[:],
                            in1=ap_of(iota_p16, 0, [[0, JK]]),
                            op=mybir.AluOpType.is_equal)

                    ald_ps = apsums.tile([128, K * Hw], F32, tag="aldps",
                                         name="ald_ps")
                    for (kk, w_, st, sp) in by_tile[t]:
                        nc.tensor.matmul(
                            out=ald_ps[:, kk * Hw:(kk + 1) * Hw],
                            lhsT=oh2_t[:, kk * 128:(kk + 1) * 128],
                            rhs=ald_t[:, w_ * Hw:(w_ + 1) * Hw],
                            start=True, stop=True)
                    ald_in = ald_ps[:]
                else:
                    ald_in = ap_of(dix_t, K * 9, [[1, K * 4]])

                if layer == 1:
                    als_ap = ap_of(rec_t[:].bitcast(F32), 14, [[RECE1 // 2, K], [1, Hw]])
                elif layer == 2:
                    als_ap = ap_of(rec_t[:].bitcast(F32), 88, [[RECE // 2, K], [1, Hw]])
                else:
                    als_ap = ap_of(rec_t[:].bitcast(F32), 65, [[RECE // 2, K], [1, Hw]])
                nc.vector.tensor_add(out=lg_t[:], in0=als_ap, in1=ald_in)
                nc.scalar.activation(out=tmp_t[:], in_=lg_t[:],
                                     func=mybir.ActivationFunctionType.Copy,
                                     scale=NEG)
                nc.vector.tensor_max(out=lg_t[:], in0=lg_t[:], in1=tmp_t[:])
                nc.scalar.activation(out=w4_t[:], in_=lg_t[:],
                                     func=mybir.ActivationFunctionType.Exp)

                if layer == 1:
                    # rhs[e, (k,f,h4)] = rec[e,(k,f,h4)] * w4[e,(k,h4)]
                    rhs_in0 = ap_of(rec_t, 0, [[RECE1, K], [1, 28]])
                    rhs_in1 = ap_of(w4_t, 0, [[4, K], [0, 7], [1, 4]])
                    nc.vector.tensor_tensor(out=rhs_t[:], in0=rhs_in0,
                                            in1=rhs_in1, op=mybir.AluOpType.mult)
                elif layer == 2:
                    rhs_in0 = ap_of(rec_t, 0, [[RECE, K], [1, 176]])
                    rhs_in1 = ap_of(w4_t, 0, [[4, K], [0, 44], [1, 4]])
                    nc.vector.tensor_tensor(out=rhs_t[:], in0=rhs_in0,
                                            in1=rhs_in1, op=mybir.AluOpType.mult)
                else:
                    # fold w into the one-hot; raw record is the matmul rhs
                    w_bcast = (ap_of(w4_t, 0, [[0, 128], [1, K]]) if USE_JOUT
                               else ap_of(w4_t, 0, [[1, K], [0, 128]]))
                    nc.vector.tensor_tensor(
                        out=oh1_t[:], in0=oh1_t[:], in1=w_bcast,
                        op=mybir.AluOpType.mult)

                for (kk, w_, st, sp) in by_tile[t]:
                    if st:
                        psum_win[w_] = psums.tile([128, rhsw], F32,
                                                  tag="agg", name="agg_ps")
                    if layer != 3:
                        rhs_ap = rhs_t[:, kk * rhsw:(kk + 1) * rhsw]
                    else:
                        rhs_ap = ap_of(rec_t, kk * RECE, [[1, 129]])
                    nc.tensor.matmul(
                        out=psum_win[w_][:],
                        lhsT=oh1_lhsT(kk),
                        rhs=rhs_ap,
                        start=st, stop=sp)
                    if sp:
                        finalize(layer, w_, psum_win.pop(w_))

        def finalize(layer, w_, ps):
            rows = LASTW if w_ == NWIN - 1 else 128
            if layer == 1:
                recip = outs.tile([128, H], F32, tag="recip1")
                nc.vector.reciprocal(out=recip[:], in_=ap_of(ps, 24, [[1, H]]))
                xn_t = outs.tile([128, 18], F16, tag="xn")
                nc.vector.tensor_tensor(
                    out=ap_of(xn_t, 0, [[6, H], [1, 6]]),
                    in0=ap_of(ps, 0, [[1, H], [4, 6]]),
                    in1=ap_of(recip, 0, [[1, H], [0, 6]]),
                    op=mybir.AluOpType.mult)
                xT_ps = nptr.tile([18, 128], F16, tag="xT", name="xT_ps")
                nc.tensor.transpose(out=xT_ps[:], in_=xn_t[:], identity=ident[:])
                xT_t = outs.tile([18, 128], F16, tag="xTs")
                nc.vector.tensor_copy(out=xT_t[:], in_=xT_ps[:])
                g_ps = npsums.tile([128, 129], F32, tag="npA", name="g1_ps")
                nc.tensor.matmul(out=g_ps[:], lhsT=xT_t[:], rhs=w0_t[:],
                                 start=True, stop=True)
                node_phase(1, w_, g_ps, rows)
            elif layer == 2:
                recip = outs.tile([128, H], F32, tag="recip2")
                nc.vector.reciprocal(out=recip[:], in_=ap_of(ps, 172, [[1, H]]))
                g_t = outs.tile([128, 129], F32, tag="g2pre")
                nc.vector.tensor_tensor(
                    out=ap_of(g_t, 0, [[F, H], [1, F]]),
                    in0=ap_of(ps, 0, [[1, H], [4, F]]),
                    in1=ap_of(recip, 0, [[1, H], [0, F]]),
                    op=mybir.AluOpType.mult)
                node_phase(2, w_, g_t, rows)
            else:
                recip = outs.tile([128, 1], F32, tag="recip3")
                nc.vector.reciprocal(out=recip[:], in_=ps[:, 128:129])
                o_t = outs.tile([128, 128], F32, tag="ofin")
                nc.vector.tensor_tensor(
                    out=o_t[:], in0=ps[:, 0:128],
                    in1=ap_of(recip, 0, [[0, 128]]),
                    op=mybir.AluOpType.mult)
                nc.sync.dma_start(out=out_d[w_ * 128:w_ * 128 + rows, :],
                                  in_=o_t[0:rows, :])

        def node_phase(layer, w_, g_in, rows):
            # g_in: layer-1 -> psum [128,129] f32 pre-activation; layer-2 -> sbuf f32
            tmp_t = outs.tile([128, 129], F32, tag="nltmp")
            gl_t = outs.tile([128, 129], F16, tag="nlgl")
            nc.scalar.activation(out=tmp_t[:], in_=g_in[:, 0:129],
                                 func=mybir.ActivationFunctionType.Copy,
                                 scale=NEG)
            nc.vector.tensor_max(out=gl_t[:], in0=g_in[:, 0:129], in1=tmp_t[:])
            t01_ps = nptr.tile([128, 256], F16, tag="t01", name="t01_ps")
            nc.tensor.transpose(out=t01_ps[:, 0:128], in_=gl_t[:, 0:128],
                                identity=ident[:])
            nc.tensor.transpose(out=t01_ps[0:1, 128:256], in_=gl_t[:, 128:129],
                                identity=ident[:])
            gT0 = outs.tile([128, 128], F16, tag="gT0")
            gT1 = outs.tile([1, 128], F16, tag="gT1")
            nc.vector.tensor_copy(out=gT0[:], in_=t01_ps[:, 0:128])
            nc.vector.tensor_copy(out=gT1[:], in_=t01_ps[0:1, 128:256])
            wa, wb = (we1_t, we1b_t) if layer == 1 else (we2_t, we2b_t)
            wcols = 135 if layer == 1 else 130
            h_ps = npsums.tile([128, wcols], F32, tag="npA", name="h_ps")
            nc.tensor.matmul(out=h_ps[:], lhsT=gT0[:], rhs=wa[:], start=True, stop=False)
            nc.tensor.matmul(out=h_ps[:], lhsT=gT1[:], rhs=wb[:], start=False, stop=True)
            rec_t = outs.tile([128, RECE], F16, tag="recslice")
            nc.vector.memset(rec_t[:], 1.0)
            if layer == 1:
                # L2 record: (f,h4) interleave of the 129 feats; ones at 172:176
                nc.vector.tensor_copy(
                    out=ap_of(rec_t, 0, [[4, F], [1, H]]),
                    in_=ap_of(h_ps, 0, [[1, F], [F, H]]))
                nc.vector.tensor_copy(
                    out=ap_of(rec_t[:].bitcast(F32), 88, [[1, H]]),
                    in_=h_ps[:, 129:132])
                nc.vector.tensor_copy(out=ald2_t[0:rows, w_ * 4:w_ * 4 + H],
                                      in_=h_ps[0:rows, 132:135])
                sa, sb = rec2_sa, rec2_sb
            else:
                nc.vector.tensor_copy(out=rec_t[:, 0:128], in_=h_ps[:, 0:128])
                nc.vector.tensor_copy(
                    out=ap_of(rec_t[:].bitcast(F32), 65, [[1, 1]]),
                    in_=h_ps[:, 128:129])
                nc.vector.tensor_copy(out=ald3_t[0:rows, w_:w_ + 1],
                                      in_=h_ps[0:rows, 129:130])
                sa, sb = rec3_sa, rec3_sb
            if w_ < WINA:
                dst_dram, off = sa, w_ * 128 * RECE
            else:
                dst_dram, off = sb, (w_ * 128 - ROWA) * RECE
            nc.sync.dma_start(
                out=bass.AP(tensor=dst_dram[:].tensor, offset=off,
                            ap=[[RECE, rows], [1, RECE]]),
                in_=rec_t[0:rows, :])

        def gather_halves(sa, sb, fa, fb):
            nc.gpsimd.collective_compute(
                "AllGather", mybir.AluOpType.bypass,
                replica_groups=[list(range(NCORE))],
                ins=[sa[:]], outs=[fa[:].rearrange("a b -> (a b)")])
            nc.gpsimd.collective_compute(
                "AllGather", mybir.AluOpType.bypass,
                replica_groups=[list(range(NCORE))],
                ins=[sb[:]], outs=[fb[:].rearrange("a b -> (a b)")])

        edge_phase(1)
        gather_halves(rec2_sa, rec2_sb, rec2_fa, rec2_fb)
        edge_phase(2)
        gather_halves(rec3_sa, rec3_sb, rec3_fa, rec3_fb)
        edge_phase(3)

    nc.compile()
    return nc


_CACHE = {}


def run(inputs, trace=False):
    x = np.asarray(inputs["x"], np.float32)
    ei = np.asarray(inputs["edge_index"]).astype(np.int64)
    W0 = np.asarray(inputs["W0"], np.float32)
    a_src0 = np.asarray(inputs["a_src0"], np.float32)
    a_dst0 = np.asarray(inputs["a_dst0"], np.float32)
    b0 = np.asarray(inputs["b0"], np.float32)
    W1 = np.asarray(inputs["W1"], np.float32)
    a_src1 = np.asarray(inputs["a_src1"], np.float32)
    a_dst1 = np.asarray(inputs["a_dst1"], np.float32)
    b1 = np.asarray(inputs["b1"], np.float32)
    W2 = np.asarray(inputs["W2"], np.float32)
    a_src2 = np.asarray(inputs["a_src2"], np.float32)
    a_dst2 = np.asarray(inputs["a_dst2"], np.float32)
    b2 = np.asarray(inputs["b2"], np.float32)
    assert np.all(b0 == 0) and np.all(b1 == 0), "nonzero hidden biases unsupported"

    loops = np.arange(N, dtype=np.int64)
    src = np.concatenate([ei[0], loops])
    dst = np.concatenate([ei[1], loops])

    skey = hash((src.tobytes(), dst.tobytes()))
    if "struct" not in _CACHE or _CACHE.get("skey") != skey:
        struct = _build_structure(src, dst)
        _CACHE.update(skey=skey, struct=struct)
        _CACHE.pop("nc", None)
    NT, sched, runs, idx_cat, d_e, d_eT, dstidx = _CACHE["struct"]
    if "nc" not in _CACHE:
        _CACHE["nc"] = _build_program(NT, sched, runs, idx_cat.shape[2])
    nc = _CACHE["nc"]

    # host precompute: layer-1 tables, extended weight matrices
    c_s0 = np.stack([W0[:, h * F:(h + 1) * F] @ a_src0[h] for h in range(H)], 1)
    c_d0 = np.stack([W0[:, h * F:(h + 1) * F] @ a_dst0[h] for h in range(H)], 1)
    al_s1 = x @ c_s0
    al_d1 = x @ c_d0
    # L1 record, (f,h4) layout: cols f*4+h = x_f (f<6) / 1.0 (f=6); als f32@14:17
    rec1 = np.zeros((N, RECE1), np.float16)
    xf = x.astype(np.float16)
    for f_ in range(6):
        for h_ in range(4):
            rec1[:, f_ * 4 + h_] = xf[:, f_]
    rec1[:, 24:28] = 1.0
    rec1[:, 28:34] = al_s1.astype(np.float32).view(np.uint16).reshape(N, 6).view(np.float16)
    # remap to table-row order
    perm = _remap_rows(np.arange(N, dtype=np.int64))
    rec1_tbl = np.zeros_like(rec1)
    rec1_tbl[perm] = rec1

    def wext(W, a_s, a_d, heads, f):
        cs = np.stack([W[:, h * f:(h + 1) * f] @ a_s[h] for h in range(heads)], 1)
        cd = np.stack([W[:, h * f:(h + 1) * f] @ a_d[h] for h in range(heads)], 1)
        return np.concatenate([W, cs, cd], axis=1).astype(np.float16)

    we1 = wext(W1, a_src1, a_dst1, 3, F)          # [129, 135]
    we2 = wext(W2, a_src2, a_dst2, 1, 128)        # [129, 130]
    w0p = np.zeros((18, 129), np.float16)         # block-diag [3x(6,43)]
    for h in range(H):
        w0p[6 * h:6 * h + 6, F * h:F * (h + 1)] = W0[:, F * h:F * (h + 1)].astype(np.float16)
    iota32 = np.arange(128, dtype=np.float32)
    iotajk = (np.arange(JK) // K).astype(np.float16)

    in_maps = []
    for c in range(NCORE):
        dsti = dstidx[c]                       # [NT, 128, K] int32
        vals = al_d1[np.maximum(dsti, 0)]      # [NT, 128, K, H] f32
        vals[dsti < 0] = 0.0
        ald1pe = np.zeros(dsti.shape + (4,), np.float16)
        ald1pe[..., :H] = vals.astype(np.float16)
        ald1pe = ald1pe.reshape(NT, 128, K * 4)
        idx_f16 = (idx_cat[c].reshape(128, NT, K * 8).transpose(1, 0, 2)
                   .copy().view(np.float16))
        dix = np.concatenate([d_e[c], idx_f16, ald1pe], axis=2)
        in_maps.append(dict(
            rec1=rec1_tbl, dix=dix, d_eT=d_eT[c],
            iota32=iota32, iotajk=iotajk, w0p=w0p,
            wext1=we1, wext2=we2))

    res = run_bass_kernel_spmd(nc, in_maps, list(range(NCORE)), trace=trace)
    out = np.concatenate([res.results[c]["out"] for c in range(NCORE)], axis=0)
    out = out + b2[None, :]
    return out.astype(np.float32), res


def kernel(**inputs) -> np.ndarray:
    out, _ = run(inputs, trace=False)
    return out


# revision 47
# speedup vs baseline: 1.1083x; 1.0379x over previous
"""3-layer GAT encoder on 8 trn2 NeuronCores (Bass/Tile).

Strategy: edge-parallel sharding by destination node block (core k owns dst
nodes [k*6250, (k+1)*6250)), so all segment ops are core-local. Per layer the
aggregation is factored as out[n,h] = (sum_e w_e * hfeat[src_e]) / (sum_e w_e)
with w_e = exp(leaky_relu(al_s[src] + al_d[dst])). Per-edge work is done in
128-edge blocks: src features come from a dma_gather of 512-byte fp16 node
records (two table halves for int16 indices), al_d[dst] is broadcast via a
onehot matmul, and the segment sum is an edge-orientation onehot matmul
accumulated in PSUM per 128-dst-node window.

Perf notes vs the first version:
 - one-hots are built in DVE 2x/4x perf modes: oh2 (dst-part orientation) via
   tensor_scalar with a per-partition f32 iota scalar (single-src 4x); oh1
   (edge-part orientation) in (j-outer, k-inner) column order against a
   materialized iota pattern so both tensor_tensor operands are unit-stride.
 - records store features interleaved (f, h4) with heads padded to 4 so the
   per-edge weight multiply has both operands unit-stride (2x mode). Layer 3
   (1 head) instead folds w into the one-hot and streams the raw record as
   the matmul rhs.
 - the node table is split into two half tensors; each half's AllGather fires
   as soon as its windows finalize, overlapping the collective with the edge
   phase tail and the next layer's start. Node->table-row order is remapped
   (half-major, then rank-major) so AllGather's rank-major concat lands rows
   exactly where the gather indices expect them.
"""
import os
import numpy as np
from contextlib import ExitStack

import concourse.bass as bass
import concourse.bacc as bacc
import concourse.tile as tile
from concourse import mybir
from concourse.bass_utils import run_bass_kernel_spmd

F16 = mybir.dt.float16
F32 = mybir.dt.float32
I16 = mybir.dt.int16

N = 50000
NCORE = 8
NLOC = N // NCORE            # 6250
NWIN = (NLOC + 127) // 128   # 49
LASTW = NLOC - 128 * (NWIN - 1)  # 106
WINA = 25                    # windows in half A
ROWA = WINA * 128            # 3200 local rows in half A
ROWB = NLOC - ROWA           # 3050 local rows in half B
HALFA = NCORE * ROWA         # 25600 table rows in half A
HALFB = NCORE * ROWB         # 24400
H, F = 3, 43
NEG = 0.2
RECE1 = 128                  # f16 record L1: [(x6+1,h4)=28, pad, al_s f32@14:17]
RECE = 256                   # f16 record L2/L3 (512B)
K = 48                       # blocks per tile
JK = K * 128                 # one-hot cols per tile


def ap_of(t, offset_elems, dims):
    base = t if isinstance(t, bass.AP) else t[:]
    return bass.AP(tensor=base.tensor, offset=base.offset + offset_elems,
                   ap=[list(base.ap[0])] + [list(d) for d in dims])


def _remap_rows(src):
    """Global node id -> table row (half-major, rank-major, local)."""
    c = src // NLOC
    r = src - c * NLOC
    return np.where(r < ROWA, c * ROWA + r, HALFA + c * ROWB + (r - ROWA))


def _build_structure(src, dst):
    """Host: shard edges by dst core / 128-window / src half, uniform block
    structure across cores. Returns per-core upload arrays + schedule."""
    core = dst // NLOC
    dst_loc = dst - core * NLOC
    win = dst_loc // 128
    de = dst_loc % 128
    row = _remap_rows(src)
    half = (row >= HALFA).astype(np.int64)

    # bucket edges per (core, win, half)
    order = np.lexsort((half, win, core))
    rc, wc, hc, dec = row[order], win[order], half[order], de[order]
    key = ((core[order] * NWIN + wc) * 2 + hc)
    uniq, starts = np.unique(key, return_index=True)
    starts = list(starts) + [len(key)]
    counts = np.zeros((NCORE, NWIN, 2), np.int64)
    seg = {}
    for i, u in enumerate(uniq):
        c_, rem = divmod(int(u), NWIN * 2)
        w_, h_ = divmod(rem, 2)
        s, e = starts[i], starts[i + 1]
        counts[c_, w_, h_] = e - s
        seg[(c_, w_, h_)] = (rc[s:e], dec[s:e])

    # uniform block counts
    B = np.maximum(np.ceil(counts / 128.0).astype(np.int64).max(axis=0), 0)
    nb_tot = int(B.sum())
    NT = (nb_tot + K - 1) // K
    pad_blocks = NT * K - nb_tot
    B[NWIN - 1, 1] += pad_blocks  # absorb tile padding into last window half-1
    nb_tot = NT * K

    # block schedule (identical for all cores): list of (win, half)
    blocks = []
    for w_ in range(NWIN):
        for h_ in range(2):
            blocks += [(w_, h_)] * int(B[w_, h_])
    assert len(blocks) == nb_tot

    # matmul schedule: (tile, k, win, start, stop)
    sched = []
    prev_w = -1
    for b, (w_, h_) in enumerate(blocks):
        st = w_ != prev_w
        sp = (b == nb_tot - 1) or (blocks[b + 1][0] != w_)
        sched.append((b // K, b % K, w_, st, sp))
        prev_w = w_

    # gather runs: per tile, maximal same-half block runs, capped length
    # >8 blocks (1024 idxs) per dma_gather call crashes the SWDGE path on HW
    RUNCAP = int(os.environ.get("GAT_RUNCAP", "8"))
    runs = []
    for t in range(NT):
        tb = blocks[t * K:(t + 1) * K]
        i = 0
        while i < len(tb):
            j = i
            while j < len(tb) and tb[j][1] == tb[i][1]:
                j += 1
            for c in range(i, j, RUNCAP):
                runs.append((t, c, min(RUNCAP, j - c), tb[i][1]))
            i = j

    # per-core uploads
    idxw = nb_tot * 128 // 16
    idx_cat = np.zeros((NCORE, 128, idxw), np.int16)
    d_e = np.full((NCORE, NT, 128, K), -1.0, np.float32)
    d_eT = np.full((NCORE, NT, K, 128), -1.0, np.float32)
    dstidx = np.full((NCORE, NT, 128, K), -1, np.int32)  # global dst node id
    # global block positions per (win, half) group, in order
    from collections import defaultdict
    gpos = defaultdict(list)
    for gb, (w_, h_) in enumerate(blocks):
        gpos[(w_, h_)].append(gb)

    for c_ in range(NCORE):
        for w_ in range(NWIN):
            for h_ in range(2):
                nb = int(B[w_, h_])
                if nb == 0:
                    continue
                r_arr, de_arr = seg.get((c_, w_, h_), (np.zeros(0, np.int64),) * 2)
                npad = nb * 128 - len(r_arr)
                loc = np.concatenate([r_arr - HALFA * h_, np.full(npad, 0, np.int64)])
                dloc = np.concatenate([de_arr, np.full(npad, -1, np.int64)])
                for b in range(nb):
                    gb = gpos[(w_, h_)][b]
                    t, kk = divmod(gb, K)
                    tok = loc[b * 128:(b + 1) * 128]
                    dl = dloc[b * 128:(b + 1) * 128]
                    dd = dl.astype(np.float32)
                    d_e[c_, t, :, kk] = dd
                    d_eT[c_, t, kk, :] = dd
                    dstidx[c_, t, :, kk] = np.where(
                        dl >= 0, c_ * NLOC + w_ * 128 + dl, -1)
                    # idx wrap: token i at [i%16, gb*8 + i//16], replicated x8
                    wrapped = tok.reshape(8, 16).T.astype(np.int16)  # [16, 8]
                    idx_cat[c_, :, gb * 8:(gb + 1) * 8] = np.tile(wrapped, (8, 1))
    return (NT, sched, runs, idx_cat, d_e.astype(np.float16),
            d_eT.astype(np.float16), dstidx)


def _build_program(NT, sched, runs, idxw):
    nc = bacc.Bacc("TRN2", target_bir_lowering=False, debug=False,
                   num_devices=NCORE, num_swdge_queues=4)
    rec1_d = nc.declare_dram_parameter("rec1", [N, RECE1], F16, isOutput=False)
    # packed per-tile sideband: [d_e (K) | idx (K*8 int16) | ald1pe (K*4, L1)]
    dix_d = nc.declare_dram_parameter("dix", [NT, 128, K * 13], F16,
                                      isOutput=False)
    deT_d = nc.declare_dram_parameter("d_eT", [NT, K, 128], F16, isOutput=False)
    iota32_d = nc.declare_dram_parameter("iota32", [128], F32, isOutput=False)
    iotajk_d = nc.declare_dram_parameter("iotajk", [JK], F16, isOutput=False)
    w0_d = nc.declare_dram_parameter("w0p", [18, 129], F16, isOutput=False)
    we1_d = nc.declare_dram_parameter("wext1", [129, 135], F16, isOutput=False)
    we2_d = nc.declare_dram_parameter("wext2", [129, 130], F16, isOutput=False)
    out_d = nc.declare_dram_parameter("out", [NLOC, 128], F32, isOutput=True)

    rec2_sa = nc.dram_tensor("rec2_sa", [ROWA * RECE], F16)
    rec2_sb = nc.dram_tensor("rec2_sb", [ROWB * RECE], F16)
    rec3_sa = nc.dram_tensor("rec3_sa", [ROWA * RECE], F16)
    rec3_sb = nc.dram_tensor("rec3_sb", [ROWB * RECE], F16)
    rec2_fa = nc.dram_tensor("rec2_fa", [HALFA, RECE], F16, addr_space="Shared")
    rec2_fb = nc.dram_tensor("rec2_fb", [HALFB, RECE], F16, addr_space="Shared")
    rec3_fa = nc.dram_tensor("rec3_fa", [HALFA, RECE], F16, addr_space="Shared")
    rec3_fb = nc.dram_tensor("rec3_fb", [HALFB, RECE], F16, addr_space="Shared")

    by_tile = {}
    for (t, kk, w_, st, sp) in sched:
        by_tile.setdefault(t, []).append((kk, w_, st, sp))
    runs_by_tile = {}
    for ri, (t, s, nb, hf) in enumerate(runs):
        runs_by_tile.setdefault(t, []).append((ri, s, nb, hf))

    with tile.TileContext(nc) as tc, ExitStack() as ctx:
        RECB = int(os.environ.get("GAT_RECB", "3"))
        SMB = int(os.environ.get("GAT_SMB", "3"))
        recs = ctx.enter_context(tc.tile_pool(name="recs", bufs=RECB))
        pool = ctx.enter_context(tc.tile_pool(name="pool", bufs=2))
        pool3 = ctx.enter_context(tc.tile_pool(name="pool3", bufs=2))
        small3 = ctx.enter_context(tc.tile_pool(name="small3", bufs=SMB))
        singles = ctx.enter_context(tc.tile_pool(name="singles", bufs=1))
        psums = ctx.enter_context(tc.tile_pool(name="psums", bufs=2, space="PSUM"))
        apsums = ctx.enter_context(tc.tile_pool(name="apsums", bufs=2, space="PSUM"))
        npsums = ctx.enter_context(tc.tile_pool(name="npsums", bufs=2, space="PSUM"))
        nptr = ctx.enter_context(tc.tile_pool(name="nptr", bufs=1, space="PSUM"))
        outs = ctx.enter_context(tc.tile_pool(name="outs", bufs=3))

        USE_TS = os.environ.get("GAT_TS", "1") == "1"
        USE_JOUT = os.environ.get("GAT_JOUT", "1") == "1"
        iota_p32 = singles.tile([128, 1], F32)
        nc.sync.dma_start(out=iota_p32[:], in_=bass.AP(
            tensor=iota32_d[:].tensor, offset=0, ap=[[1, 128], [0, 1]]))
        iota_jk = singles.tile([128, JK], F16)
        nc.sync.dma_start(out=iota_jk[:], in_=bass.AP(
            tensor=iotajk_d[:].tensor, offset=0, ap=[[0, 128], [1, JK]]))
        if not USE_TS:
            # per-partition iota as f16: iotajk[j*K] = j
            iota_p16 = singles.tile([128, 1], F16)
            nc.sync.dma_start(out=iota_p16[:], in_=bass.AP(
                tensor=iotajk_d[:].tensor, offset=0, ap=[[K, 128], [0, 1]]))
        if not USE_JOUT:
            # row iota [p, j] = j: iotajk[j*K] = j read with col stride K
            iota_row = singles.tile([128, 128], F16)
            nc.sync.dma_start(out=iota_row[:], in_=bass.AP(
                tensor=iotajk_d[:].tensor, offset=0, ap=[[0, 128], [K, 128]]))
        from concourse.masks import make_identity
        ident = singles.tile([128, 128], F16)
        make_identity(nc, ident[:])
        w0_t = singles.tile([18, 129], F16)
        nc.sync.dma_start(out=w0_t[:], in_=w0_d[:])
        we1_t = singles.tile([128, 135], F16)
        nc.sync.dma_start(out=we1_t[:], in_=we1_d[0:128, :])
        we1b_t = singles.tile([1, 135], F16)
        nc.sync.dma_start(out=we1b_t[:], in_=we1_d[128:129, :])
        we2_t = singles.tile([128, 130], F16)
        nc.sync.dma_start(out=we2_t[:], in_=we2_d[0:128, :])
        we2b_t = singles.tile([1, 130], F16)
        nc.sync.dma_start(out=we2b_t[:], in_=we2_d[128:129, :])

        # al_d stages for layers 2/3: [p, w*H4] f16 ; node (w,p) at col w*H4
        # (heads padded to 4 so the whole logits pipeline is (k,h4)-wide).
        # Layer 1's al_d is host-precomputed per edge (ald1pe_d).
        H4 = 4
        ald2_t = singles.tile([128, NWIN * H4], F16)
        nc.vector.memset(ald2_t[:], 0.0)
        ald3_t = singles.tile([128, NWIN], F16)
        nc.vector.memset(ald3_t[:], 0.0)

        gather_ctr = [0]  # DMASW sems pair queues by emission order (mod 8/4)

        def edge_phase(layer):
            Hw = 4 if layer < 3 else 1   # padded head width of the w pipeline
            rece = RECE1 if layer == 1 else RECE
            # rhs column width per block (f,h4-interleaved for L1/L2)
            rhsw = 28 if layer == 1 else (176 if layer == 2 else 129)
            ald_t = (None, ald2_t, ald3_t)[layer - 1]
            psum_win = {}
            dixw = K * 13 if layer == 1 else K * 9
            for t in range(NT):
                rec_t = recs.tile([128, K * rece], F16, tag="rec")
                dix_t = small3.tile([128, dixw], F16, tag="dix")
                oh1_t = pool3.tile([128, JK], F16, tag="oh1")
                lg_t = small3.tile([128, K * Hw], F32, tag="lg")
                tmp_t = small3.tile([128, K * Hw], F32, tag="tmp")
                w4_t = small3.tile([128, K * Hw], F16, tag="w")
                if layer != 3:
                    rhs_t = pool3.tile([128, K * rhsw], F16, tag="rhs")

                # packed sideband: de [0:K], idx [K:K*9], ald1pe [K*9:K*13]
                nc.sync.dma_start(out=dix_t[:], in_=bass.AP(
                    tensor=dix_d[:].tensor, offset=t * 128 * K * 13,
                    ap=[[K * 13, 128], [1, dixw]]))
                de_t = dix_t
                if layer == 2:
                    deT_t = pool.tile([128, JK], F16, tag="deT")
                    nc.sync.dma_start(out=deT_t[:], in_=bass.AP(
                        tensor=deT_d[:].tensor, offset=t * JK,
                        ap=[[0, 128], [1, JK]]))
                elif layer == 3:
                    # oh2 built on-chip by transposing oh1 blocks (no deT DMA)
                    oh2s_t = pool.tile([128, JK], F16, tag="deT")

                for (ri, s, nb, hf) in runs_by_tile[t]:
                    n_idx = nb * 128
                    if layer == 1:
                        in_ap = rec1_d[HALFA:, :] if hf else rec1_d[0:HALFA, :]
                    elif layer == 2:
                        in_ap = rec2_fb[:] if hf else rec2_fa[:]
                    else:
                        in_ap = rec3_fb[:] if hf else rec3_fa[:]
                    base = rec_t[:]
                    out_ap = bass.AP(
                        tensor=base.tensor, offset=base.offset + s * rece,
                        ap=[list(base.ap[0]), [rece, nb], [1, rece]])
                    nc.gpsimd.dma_gather(
                        out_ap=out_ap, in_ap=in_ap,
                        idxs_ap=ap_of(dix_t[:].bitcast(I16), K + s * 8,
                                      [[1, nb * 8]]),
                        num_idxs=n_idx, num_idxs_reg=n_idx, elem_size=rece,
                        queue_num=gather_ctr[0] % 4)
                    gather_ctr[0] += 1

                # one-hot, edge partitions: oh1[e, j*K+k] = (de[e,k] == j)
                # (or [e, k*128+j] when USE_JOUT is off)
                if USE_JOUT:
                    nc.vector.tensor_tensor(
                        out=oh1_t[:],
                        in0=ap_of(de_t, 0, [[0, 128], [1, K]]),
                        in1=iota_jk[:],
                        op=mybir.AluOpType.is_equal)

                    def oh1_lhsT(kk):
                        return ap_of(oh1_t, kk, [[K, 128]])
                else:
                    nc.vector.tensor_tensor(
                        out=oh1_t[:],
                        in0=ap_of(de_t, 0, [[1, K], [0, 128]]),
                        in1=ap_of(iota_row, 0, [[0, K], [1, 128]]),
                        op=mybir.AluOpType.is_equal)

                    def oh1_lhsT(kk):
                        return oh1_t[:, kk * 128:(kk + 1) * 128]
                if layer > 1:
                    # one-hot, dst partitions: oh2[p, k*128+e] = (de[k,e] == p)
                    if layer == 2:
                        oh2_t = deT_t
                        if USE_TS:
                            nc.vector.tensor_scalar(
                                deT_t[:], deT_t[:], iota_p32[:], None,
                                mybir.AluOpType.is_equal)
                        else:
                            nc.vector.tensor_tensor(
                                out=deT_t[:], in0=deT_t[:],
                                in1=ap_of(iota_p16, 0, [[0, JK]]),
                                op=mybir.AluOpType.is_equal)
                    else:
                        # transpose oh1 blocks on the PE, 8 blocks per bank
                        oh2_t = oh2s_t
                        for g in range(K // 8):
                            scr = nptr.tile([128, 1024], F16, tag="scr",
                                            name="ohT_ps")
                            for j8 in range(8):
                                kk = g * 8 + j8
                                nc.tensor.transpose(
                                    out=scr[:, j8 * 128:(j8 + 1) * 128],
                                    in_=oh1_lhsT(kk), identity=ident[:])
                            nc.vector.tensor_copy(
                                out=oh2s_t[:, g * 1024:(g + 1) * 1024],
                                in_=scr[:])

                    ald_ps = apsums.tile([128, K * Hw], F32, tag="aldps",
                                         name="ald_ps")
                    for (kk, w_, st, sp) in by_tile[t]:
                        nc.tensor.matmul(
                            out=ald_ps[:, kk * Hw:(kk + 1) * Hw],
                            lhsT=oh2_t[:, kk * 128:(kk + 1) * 128],
                            rhs=ald_t[:, w_ * Hw:(w_ + 1) * Hw],
                            start=True, stop=True)
                    ald_in = ald_ps[:]
                else:
                    ald_in = ap_of(dix_t, K * 9, [[1, K * 4]])

                if layer == 1:
                    als_ap = ap_of(rec_t[:].bitcast(F32), 14, [[RECE1 // 2, K], [1, Hw]])
                elif layer == 2:
                    als_ap = ap_of(rec_t[:].bitcast(F32), 88, [[RECE // 2, K], [1, Hw]])
                else:
                    als_ap = ap_of(rec_t[:].bitcast(F32), 65, [[RECE // 2, K], [1, Hw]])
                nc.vector.tensor_add(out=lg_t[:], in0=als_ap, in1=ald_in)
                nc.scalar.activation(out=tmp_t[:], in_=lg_t[:],
                                     func=mybir.ActivationFunctionType.Copy,
                                     scale=NEG)
                nc.vector.tensor_max(out=lg_t[:], in0=lg_t[:], in1=tmp_t[:])
                nc.scalar.activation(out=w4_t[:], in_=lg_t[:],
                                     func=mybir.ActivationFunctionType.Exp)

                if layer == 1:
                    # rhs[e, (k,f,h4)] = rec[e,(k,f,h4)] * w4[e,(k,h4)]
                    rhs_in0 = ap_of(rec_t, 0, [[RECE1, K], [1, 28]])
                    rhs_in1 = ap_of(w4_t, 0, [[4, K], [0, 7], [1, 4]])
                    nc.vector.tensor_tensor(out=rhs_t[:], in0=rhs_in0,
                                            in1=rhs_in1, op=mybir.AluOpType.mult)
                elif layer == 2:
                    rhs_in0 = ap_of(rec_t, 0, [[RECE, K], [1, 176]])
                    rhs_in1 = ap_of(w4_t, 0, [[4, K], [0, 44], [1, 4]])
                    nc.vector.tensor_tensor(out=rhs_t[:], in0=rhs_in0,
                                            in1=rhs_in1, op=mybir.AluOpType.mult)
                else:
                    # fold w into the one-hot; raw record is the matmul rhs
                    w_bcast = (ap_of(w4_t, 0, [[0, 128], [1, K]]) if USE_JOUT
                               else ap_of(w4_t, 0, [[1, K], [0, 128]]))
                    nc.vector.tensor_tensor(
                        out=oh1_t[:], in0=oh1_t[:], in1=w_bcast,
                        op=mybir.AluOpType.mult)

                for (kk, w_, st, sp) in by_tile[t]:
                    if st:
                        psum_win[w_] = psums.tile([128, rhsw], F32,
                                                  tag="agg", name="agg_ps")
                    if layer != 3:
                        rhs_ap = rhs_t[:, kk * rhsw:(kk + 1) * rhsw]
                    else:
                        rhs_ap = ap_of(rec_t, kk * RECE, [[1, 129]])
                    nc.tensor.matmul(
                        out=psum_win[w_][:],
                        lhsT=oh1_lhsT(kk),
                        rhs=rhs_ap,
                        start=st, stop=sp)
                    if sp:
                        finalize(layer, w_, psum_win.pop(w_))

        def finalize(layer, w_, ps):
            rows = LASTW if w_ == NWIN - 1 else 128
            if layer == 1:
                recip = outs.tile([128, H], F32, tag="recip1")
                nc.vector.reciprocal(out=recip[:], in_=ap_of(ps, 24, [[1, H]]))
                xn_t = outs.tile([128, 18], F16, tag="xn")
                nc.vector.tensor_tensor(
                    out=ap_of(xn_t, 0, [[6, H], [1, 6]]),
                    in0=ap_of(ps, 0, [[1, H], [4, 6]]),
                    in1=ap_of(recip, 0, [[1, H], [0, 6]]),
                    op=mybir.AluOpType.mult)
                xT_ps = nptr.tile([18, 128], F16, tag="xT", name="xT_ps")
                nc.tensor.transpose(out=xT_ps[:], in_=xn_t[:], identity=ident[:])
                xT_t = outs.tile([18, 128], F16, tag="xTs")
                nc.vector.tensor_copy(out=xT_t[:], in_=xT_ps[:])
                g_ps = npsums.tile([128, 129], F32, tag="npA", name="g1_ps")
                nc.tensor.matmul(out=g_ps[:], lhsT=xT_t[:], rhs=w0_t[:],
                                 start=True, stop=True)
                node_phase(1, w_, g_ps, rows)
            elif layer == 2:
                recip = outs.tile([128, H], F32, tag="recip2")
                nc.vector.reciprocal(out=recip[:], in_=ap_of(ps, 172, [[1, H]]))
                g_t = outs.tile([128, 129], F32, tag="g2pre")
                nc.vector.tensor_tensor(
                    out=ap_of(g_t, 0, [[F, H], [1, F]]),
                    in0=ap_of(ps, 0, [[1, H], [4, F]]),
                    in1=ap_of(recip, 0, [[1, H], [0, F]]),
                    op=mybir.AluOpType.mult)
                node_phase(2, w_, g_t, rows)
            else:
                recip = outs.tile([128, 1], F32, tag="recip3")
                nc.vector.reciprocal(out=recip[:], in_=ps[:, 128:129])
                o_t = outs.tile([128, 128], F32, tag="ofin")
                nc.vector.tensor_tensor(
                    out=o_t[:], in0=ps[:, 0:128],
                    in1=ap_of(recip, 0, [[0, 128]]),
                    op=mybir.AluOpType.mult)
                nc.sync.dma_start(out=out_d[w_ * 128:w_ * 128 + rows, :],
                                  in_=o_t[0:rows, :])

        def node_phase(layer, w_, g_in, rows):
            # g_in: layer-1 -> psum [128,129] f32 pre-activation; layer-2 -> sbuf f32
            tmp_t = outs.tile([128, 129], F32, tag="nltmp")
            gl_t = outs.tile([128, 129], F16, tag="nlgl")
            nc.scalar.activation(out=tmp_t[:], in_=g_in[:, 0:129],
                                 func=mybir.ActivationFunctionType.Copy,
                                 scale=NEG)
            nc.vector.tensor_max(out=gl_t[:], in0=g_in[:, 0:129], in1=tmp_t[:])
            t01_ps = nptr.tile([128, 1024], F16, tag="scr", name="t01_ps")
            nc.tensor.transpose(out=t01_ps[:, 0:128], in_=gl_t[:, 0:128],
                                identity=ident[:])
            nc.tensor.transpose(out=t01_ps[0:1, 128:256], in_=gl_t[:, 128:129],
                                identity=ident[:])
            gT0 = outs.tile([128, 128], F16, tag="gT0")
            gT1 = outs.tile([1, 128], F16, tag="gT1")
            nc.vector.tensor_copy(out=gT0[:], in_=t01_ps[:, 0:128])
            nc.vector.tensor_copy(out=gT1[:], in_=t01_ps[0:1, 128:256])
            wa, wb = (we1_t, we1b_t) if layer == 1 else (we2_t, we2b_t)
            wcols = 135 if layer == 1 else 130
            h_ps = npsums.tile([128, wcols], F32, tag="npA", name="h_ps")
            nc.tensor.matmul(out=h_ps[:], lhsT=gT0[:], rhs=wa[:], start=True, stop=False)
            nc.tensor.matmul(out=h_ps[:], lhsT=gT1[:], rhs=wb[:], start=False, stop=True)
            rec_t = outs.tile([128, RECE], F16, tag="recslice")
            nc.vector.memset(rec_t[:], 1.0)
            if layer == 1:
                # L2 record: (f,h4) interleave of the 129 feats; ones at 172:176
                nc.vector.tensor_copy(
                    out=ap_of(rec_t, 0, [[4, F], [1, H]]),
                    in_=ap_of(h_ps, 0, [[1, F], [F, H]]))
                nc.vector.tensor_copy(
                    out=ap_of(rec_t[:].bitcast(F32), 88, [[1, H]]),
                    in_=h_ps[:, 129:132])
                nc.vector.tensor_copy(out=ald2_t[0:rows, w_ * 4:w_ * 4 + H],
                                      in_=h_ps[0:rows, 132:135])
                sa, sb = rec2_sa, rec2_sb
            else:
                nc.vector.tensor_copy(out=rec_t[:, 0:128], in_=h_ps[:, 0:128])
                nc.vector.tensor_copy(
                    out=ap_of(rec_t[:].bitcast(F32), 65, [[1, 1]]),
                    in_=h_ps[:, 128:129])
                nc.vector.tensor_copy(out=ald3_t[0:rows, w_:w_ + 1],
                                      in_=h_ps[0:rows, 129:130])
                sa, sb = rec3_sa, rec3_sb
            if w_ < WINA:
                dst_dram, off = sa, w_ * 128 * RECE
            else:
                dst_dram, off = sb, (w_ * 128 - ROWA) * RECE
            nc.sync.dma_start(
                out=bass.AP(tensor=dst_dram[:].tensor, offset=off,
                            ap=[[RECE, rows], [1, RECE]]),
                in_=rec_t[0:rows, :])

        def gather_halves(sa, sb, fa, fb):
            nc.gpsimd.collective_compute(
                "AllGather", mybir.AluOpType.bypass,
                replica_groups=[list(range(NCORE))],
                ins=[sa[:]], outs=[fa[:].rearrange("a b -> (a b)")])
            nc.gpsimd.collective_compute(
                "AllGather", mybir.AluOpType.bypass,
                replica_groups=[list(range(NCORE))],
                ins=[sb[:]], outs=[fb[:].rearrange("a b -> (a b)")])

        edge_phase(1)
        gather_halves(rec2_sa, rec2_sb, rec2_fa, rec2_fb)
        edge_phase(2)
        gather_halves(rec3_sa, rec3_sb, rec3_fa, rec3_fb)
        edge_phase(3)

    nc.compile()
    return nc


_CACHE = {}


def run(inputs, trace=False):
    x = np.asarray(inputs["x"], np.float32)
    ei = np.asarray(inputs["edge_index"]).astype(np.int64)
    W0 = np.asarray(inputs["W0"], np.float32)
    a_src0 = np.asarray(inputs["a_src0"], np.float32)
    a_dst0 = np.asarray(inputs["a_dst0"], np.float32)
    b0 = np.asarray(inputs["b0"], np.float32)
    W1 = np.asarray(inputs["W1"], np.float32)
    a_src1 = np.asarray(inputs["a_src1"], np.float32)
    a_dst1 = np.asarray(inputs["a_dst1"], np.float32)
    b1 = np.asarray(inputs["b1"], np.float32)
    W2 = np.asarray(inputs["W2"], np.float32)
    a_src2 = np.asarray(inputs["a_src2"], np.float32)
    a_dst2 = np.asarray(inputs["a_dst2"], np.float32)
    b2 = np.asarray(inputs["b2"], np.float32)
    assert np.all(b0 == 0) and np.all(b1 == 0), "nonzero hidden biases unsupported"

    loops = np.arange(N, dtype=np.int64)
    src = np.concatenate([ei[0], loops])
    dst = np.concatenate([ei[1], loops])

    skey = hash((src.tobytes(), dst.tobytes()))
    if "struct" not in _CACHE or _CACHE.get("skey") != skey:
        struct = _build_structure(src, dst)
        _CACHE.update(skey=skey, struct=struct)
        _CACHE.pop("nc", None)
    NT, sched, runs, idx_cat, d_e, d_eT, dstidx = _CACHE["struct"]
    if "nc" not in _CACHE:
        _CACHE["nc"] = _build_program(NT, sched, runs, idx_cat.shape[2])
    nc = _CACHE["nc"]

    # host precompute: layer-1 tables, extended weight matrices
    c_s0 = np.stack([W0[:, h * F:(h + 1) * F] @ a_src0[h] for h in range(H)], 1)
    c_d0 = np.stack([W0[:, h * F:(h + 1) * F] @ a_dst0[h] for h in range(H)], 1)
    al_s1 = x @ c_s0
    al_d1 = x @ c_d0
    # L1 record, (f,h4) layout: cols f*4+h = x_f (f<6) / 1.0 (f=6); als f32@14:17
    rec1 = np.zeros((N, RECE1), np.float16)
    xf = x.astype(np.float16)
    for f_ in range(6):
        for h_ in range(4):
            rec1[:, f_ * 4 + h_] = xf[:, f_]
    rec1[:, 24:28] = 1.0
    rec1[:, 28:34] = al_s1.astype(np.float32).view(np.uint16).reshape(N, 6).view(np.float16)
    # remap to table-row order
    perm = _remap_rows(np.arange(N, dtype=np.int64))
    rec1_tbl = np.zeros_like(rec1)
    rec1_tbl[perm] = rec1

    def wext(W, a_s, a_d, heads, f):
        cs = np.stack([W[:, h * f:(h + 1) * f] @ a_s[h] for h in range(heads)], 1)
        cd = np.stack([W[:, h * f:(h + 1) * f] @ a_d[h] for h in range(heads)], 1)
        return np.concatenate([W, cs, cd], axis=1).astype(np.float16)

    we1 = wext(W1, a_src1, a_dst1, 3, F)          # [129, 135]
    we2 = wext(W2, a_src2, a_dst2, 1, 128)        # [129, 130]
    w0p = np.zeros((18, 129), np.float16)         # block-diag [3x(6,43)]
    for h in range(H):
        w0p[6 * h:6 * h + 6, F * h:F * (h + 1)] = W0[:, F * h:F * (h + 1)].astype(np.float16)
    iota32 = np.arange(128, dtype=np.float32)
    iotajk = (np.arange(JK) // K).astype(np.float16)

    in_maps = []
    for c in range(NCORE):
        dsti = dstidx[c]                       # [NT, 128, K] int32
        vals = al_d1[np.maximum(dsti, 0)]      # [NT, 128, K, H] f32
        vals[dsti < 0] = 0.0
        ald1pe = np.zeros(dsti.shape + (4,), np.float16)
        ald1pe[..., :H] = vals.astype(np.float16)
        ald1pe = ald1pe.reshape(NT, 128, K * 4)
        idx_f16 = (idx_cat[c].reshape(128, NT, K * 8).transpose(1, 0, 2)
                   .copy().view(np.float16))
        dix = np.concatenate([d_e[c], idx_f16, ald1pe], axis=2)
        in_maps.append(dict(
            rec1=rec1_tbl, dix=dix, d_eT=d_eT[c],
            iota32=iota32, iotajk=iotajk, w0p=w0p,
            wext1=we1, wext2=we2))

    res = run_bass_kernel_spmd(nc, in_maps, list(range(NCORE)), trace=trace)
    out = np.concatenate([res.results[c]["out"] for c in range(NCORE)], axis=0)
    out = out + b2[None, :]
    return out.astype(np.float32), res


def kernel(**inputs) -> np.ndarray:
    out, _ = run(inputs, trace=False)
    return out
